# revision 29
# baseline (speedup 1.0000x reference)
"""Trainium2 Bass kernel for qk-layernorm attention (dense transformer block).

Sharding: 8 cores = 2 batches x 4 head-groups (4 heads each).  Each core
computes qkv projection (its heads only), qk-layernorm, attention, and a
partial output projection for its head slice; the host sums the 4 partials
per batch and adds b_proj.

Key speed choices vs the v1 kernel:
 - Wq/Wk head-slices are mean-centered on the host, so projected q/k have
   exactly zero mean over head-dim: the LN mean path (stats matmul, square,
   subtract) disappears; only E[q^2] remains (one matmul per pair/type).
 - rsqrt via ACT Sqrt + DVE reciprocal_approx_fast (the plain DVE
   reciprocal is ~9ns/elem and dominated the old DVE timeline).
 - Attention runs as one continuous software-pipelined stream over
   (n-block 512, pair, m-tile): PE does S(g) then attn@v(g-1) while ACT
   does exp(g-1); psS is double-buffered (2x2 PSUM banks) so PE never
   waits on the exp read.  Out-projection tiles of block i inject into
   block i+1's stream to fill PE gaps.
 - Softmax denominator: ones-column in the attn@v weights accumulates
   sum(exp) in an extra PSUM row; a K=1 matmul broadcasts both heads'
   denominator rows into one [128,512] bank, one reciprocal_approx_fast,
   two aligned DVE muls.  No DMA partition-shift: pair-head-1's attn@v
   writes PSUM partitions 63:128 directly ([ones|v1] weights, out AP
   offset 63).
 - Split weight/x DMAs so the first proj matmul starts ~3us in.
"""

import numpy as np

DIM = 1024
HEADS = 16
HD = 64
B = 2
N = 2048
EPS = 1e-6
N_CORES = 8
HEADS_PER_CORE = 4
PAIRS = 2          # head pairs per core
CC = 8             # contraction chunks of 128 over DIM
NT = N // 128      # 16 n/m tiles
NCH = N // 512     # 4 chunks of 512
NB = N // 512      # attention n-blocks of 512
VSTR = 260         # v_sb cols per nt: 2 pairs x [v0|ones|v1|ones] (130 each)
SCALE = HD ** -0.5

_prog_cache = {}


def _build_program():
    import concourse.bass as bass
    import concourse.tile as tile
    from concourse import mybir, bacc

    F32 = mybir.dt.float32
    F32R = mybir.dt.float32r
    BF16 = mybir.dt.bfloat16
    Act = mybir.ActivationFunctionType

    nc = bacc.Bacc("TRN2", target_bir_lowering=False, debug=False,
                   num_devices=N_CORES)

    # ---- DRAM I/O ----
    xT_d = nc.dram_tensor("xT", [128, NCH * CC * 512], BF16, kind="ExternalInput").ap()
    wqk_d = nc.dram_tensor("wqk", [128, CC * 512], BF16, kind="ExternalInput").ap()
    wv_d = nc.dram_tensor("wv", [128, CC * 256], BF16, kind="ExternalInput").ap()
    wp_d = nc.dram_tensor("wp", [128, 2 * DIM], BF16, kind="ExternalInput").ap()
    ssq_q_d = nc.dram_tensor("ssq_q", [128, 128], BF16, kind="ExternalInput").ap()
    ssq_k_d = nc.dram_tensor("ssq_k", [128, 128], BF16, kind="ExternalInput").ap()
    ones_d = nc.dram_tensor("ones", [128, 64], BF16, kind="ExternalInput").ap()
    ident_d = nc.dram_tensor("ident", [128, 128], F32R, kind="ExternalInput").ap()
    y_d = nc.dram_tensor("y", [N, DIM], F32, kind="ExternalOutput").ap()


    with tile.TileContext(nc) as tc:
        with tc.tile_pool(name="wts", bufs=1) as wts, \
             tc.tile_pool(name="persist", bufs=1) as persist:
            # 2-cc granular weight tiles; loads split across both HWDGE
            # queues (sync + scalar) so the first matmul gates on 0.5MB
            wqk_t = [wts.tile([128, 1024], BF16, tag=f"wqk{k}",
                              name=f"wqk{k}") for k in range(4)]
            wv_t = [wts.tile([128, 512], BF16, tag=f"wv{k}",
                             name=f"wv{k}") for k in range(4)]
            nc.sync.dma_start(wqk_t[0][:], wqk_d[:, 0:1024])

            # persistent SBUF tensors
            qk = [persist.tile([128, N], BF16, tag=f"qk{i}", name=f"qk{i}")
                  for i in range(4)]           # 0,1: q pairs; 2,3: k pairs
            vT = [persist.tile([128, N], F32R, tag=f"vT{p}", name=f"vT{p}")
                  for p in range(PAIRS)]
            v_sb = persist.tile([128, NT * VSTR], BF16)
            onorm = [persist.tile([128, N], BF16, tag=f"on{p}", name=f"on{p}")
                     for p in range(PAIRS)]

            wp = wts.tile([128, 2 * DIM], BF16)
            ssq = [wts.tile([128, 128], BF16, tag=f"ssq{t}", name=f"ssq{t}")
                   for t in range(2)]
            ones = wts.tile([128, 64], BF16)
            ident = wts.tile([128, 128], F32R)
            epsb = wts.tile([128, 1], F32)

            def misc_dmas():
                nc.sync.dma_start(wp[:], wp_d[:])
                nc.sync.dma_start(ssq[0][:], ssq_q_d[:])
                nc.sync.dma_start(ssq[1][:], ssq_k_d[:])
                nc.sync.dma_start(ones[:], ones_d[:])
                nc.sync.dma_start(ident[:], ident_d[:])
                # ones columns of v_sb (den accumulators): col 64 of each
                # 65-wide [v | ones] block
                v_ones = v_sb[:].rearrange(
                    "p (nt q c) -> p nt q c", q=4, c=65)[:, :, :, 64:65]
                nc.sync.dma_start(
                    v_ones,
                    ones_d.rearrange("p (nt q) -> p nt q",
                                     q=4)[:, :, :, None])
                nc.gpsimd.memset(epsb[:], EPS)

            # ================= Phase 1: qkv projection + qk-LN ============
            # jobs: (dest tile, kind, col offset within per-cc weight block)
            jobs = [(qk[0], 'qk', 0), (qk[1], 'qk', 128),
                    (qk[2], 'qk', 256), (qk[3], 'qk', 384),
                    (vT[0], 'v', 0), (vT[1], 'v', 128)]

            with tc.tile_pool(name="xp", bufs=2) as xp, \
                 tc.tile_pool(name="t1", bufs=2) as t1p, \
                 tc.tile_pool(name="ps1", bufs=1, space="PSUM") as ps1:

                T_pend = []   # deferred v-transpose closures

                def proj_mms(nch):
                    sl = slice(nch * 512, (nch + 1) * 512)
                    xb = nch * CC * 512
                    # x tiles at 2-cc granularity, DMAs alternating between
                    # the two HWDGE queues (sync / scalar)
                    xt_t = []
                    for k in range(4):
                        xt = xp.tile([128, 1024], BF16, tag=f"xt{k}",
                                     name=f"xt{k}")
                        eng = nc.scalar if k % 2 == 0 else nc.sync
                        eng.dma_start(
                            xt[:], xT_d[:, xb + k * 1024:xb + (k + 1) * 1024])
                        xt_t.append(xt)
                        if nch == 0 and k > 0:
                            nc.sync.dma_start(
                                wqk_t[k][:],
                                wqk_d[:, k * 1024:(k + 1) * 1024])
                        if nch == 0:
                            nc.scalar.dma_start(
                                wv_t[k][:], wv_d[:, k * 512:(k + 1) * 512])
                    accs = [ps1.tile([128, 512], F32, tag=f"acc{j}",
                                     name=f"acc{j}") for j in range(6)]

                    def one_mm(cc, j):
                        dest, kind, off = jobs[j]
                        xsl = xt_t[cc // 2][:, (cc % 2) * 512:
                                            (cc % 2 + 1) * 512]
                        if kind == 'qk':
                            wsl = wqk_t[cc // 2][:, (cc % 2) * 512 + off:
                                                 (cc % 2) * 512 + off + 128]
                        else:
                            wsl = wv_t[cc // 2][:, (cc % 2) * 256 + off:
                                                (cc % 2) * 256 + off + 128]
                        nc.tensor.matmul(accs[j][:], wsl, xsl,
                                         start=(cc == 0),
                                         stop=(cc == CC - 1))

                    if nch == 0:
                        # qk jobs first: first matmuls gate only on
                        # wqkA+xtA while wv streams in
                        for cc in range(CC):
                            for j in range(4):
                                one_mm(cc, j)
                        for cc in range(CC):
                            for j in (4, 5):
                                one_mm(cc, j)
                    else:
                        for cc in range(CC):
                            for j in range(6):
                                one_mm(cc, j)
                            # fill PE slack with deferred v-transposes
                            # (cc<=5: keep the aux psum tag clear for psq)
                            if 2 <= cc <= 5:
                                for _ in range(2):
                                    if T_pend:
                                        T_pend.pop(0)()
                    # drain PSUM -> SBUF, split across DVE and ACT
                    for j, (dest, kind, off) in enumerate(jobs):
                        if j % 2 == 0:
                            nc.vector.tensor_copy(dest[:, sl], accs[j][:])
                        else:
                            nc.scalar.copy(dest[:, sl], accs[j][:])

                def stats_tail(nch):
                    # E[q^2] -> rsqrt -> scale, per (pair, q/k); plus v
                    # transposes into v_sb for this n-chunk
                    sl = slice(nch * 512, (nch + 1) * 512)
                    for p in range(PAIRS):
                        for t in range(2):      # 0 = q, 1 = k
                            src = qk[2 * t + p]
                            sqc = t1p.tile([128, 512], BF16, tag="sqc",
                                           name="sqc")
                            nc.gpsimd.tensor_mul(sqc[:], src[:, sl],
                                                 src[:, sl])
                            psq = ps1.tile([128, 512], F32, tag="aux",
                                           bufs=2, name="psq")
                            nc.tensor.matmul(psq[:], ssq[t][:], sqc[:],
                                             start=True, stop=True)
                            sd = t1p.tile([128, 512], F32, tag="sd",
                                          name="sd")
                            nc.scalar.activation(sd[:], psq[:], Act.Sqrt,
                                                 bias=epsb[:])
                            rs = t1p.tile([128, 512], F32, tag="rs",
                                          name="rs")
                            nc.vector.reciprocal_approx_fast(rs[:], sd[:])
                            nc.vector.tensor_mul(src[:, sl], src[:, sl],
                                                 rs[:])
                    for j in range(4):
                        nt = nch * 4 + j
                        for p in range(PAIRS):
                            def T(nt=nt, p=p, tag="aux", bufs=2,
                                  eng=None):
                                ptr = ps1.tile([128, 128], F32R, tag=tag,
                                               bufs=bufs, name="ptr")
                                nc.tensor.transpose(
                                    ptr[:],
                                    vT[p][:, nt * 128:(nt + 1) * 128],
                                    ident[:])
                                base = nt * VSTR + p * 130
                                dst = v_sb[:, base:base + 130].rearrange(
                                    "p (b c) -> p b c", c=65)[:, :, 0:64]
                                srcv = ptr[:].rearrange(
                                    "p (b c) -> p b c", c=64)
                                if eng == 'vector':
                                    nc.vector.tensor_copy(dst, srcv)
                                else:
                                    nc.scalar.copy(dst, srcv)
                            T_pend.append(T)

                for nch in range(NCH):
                    proj_mms(nch)
                    if nch == 0:
                        misc_dmas()
                    if nch > 0:
                        stats_tail(nch - 1)
                stats_tail(NCH - 1)
                # remaining transposes fill the PE window while the last
                # k-chunk's LN chain completes on ACT/DVE; rotate over the
                # freed acc banks so ptr copies never stall the PE
                ti = 0
                while T_pend:
                    T_pend.pop(0)(tag=f"acc{ti % 6}", bufs=1,
                                  eng='vector' if ti % 2 else 'scalar')
                    ti += 1

            # ================= Phase 3: attention =================
            with tc.tile_pool(name="p3", bufs=1) as p3, \
                 tc.tile_pool(name="ps3", bufs=1, space="PSUM") as ps3:

                pend = []        # deferred closures: ('av'|'den', fn)
                proj_pend = []   # nt tiles ready for out-projection

                def pop_pend():
                    # one av per iteration keeps the software-pipeline lag
                    # constant; block-end den closures ride along right
                    # after the av they follow
                    if pend:
                        pend.pop(0)[1]()
                    while pend and pend[0][0] == 'den':
                        pend.pop(0)[1]()

                def emit_proj_tile(nt, tag="aux1024", bufs=1, split=False):
                    py = ps3.tile([128, 1024], F32, tag=tag, bufs=bufs,
                                  name="py")
                    yt = p3.tile([128, 1024], F32, tag="yt", bufs=3,
                                 name="yt")
                    for oc in range(2):
                        for p in range(PAIRS):
                            nc.tensor.matmul(
                                py[:, oc * 512:(oc + 1) * 512],
                                onorm[p][:, nt * 128:(nt + 1) * 128],
                                wp[:, p * 1024 + oc * 512:
                                   p * 1024 + (oc + 1) * 512],
                                start=(p == 0), stop=(p == PAIRS - 1))
                        if split:
                            # drain per oc-half so the last tile's copy and
                            # DMA overlap the other half's matmuls
                            osl = slice(oc * 512, (oc + 1) * 512)
                            nc.vector.tensor_copy(yt[:, osl], py[:, osl])
                            nc.sync.dma_start(
                                y_d[nt * 128:(nt + 1) * 128, osl],
                                yt[:, osl])
                    if not split:
                        nc.vector.tensor_copy(yt[:], py[:])
                        nc.sync.dma_start(y_d[nt * 128:(nt + 1) * 128, :],
                                          yt[:])

                def make_den_norm(poh, p, nh, last=False):
                    nsl = slice(nh * 512, (nh + 1) * 512)

                    def den_norm():
                        pohA, pohB = poh['A'], poh['B']
                        # rows 0:64 out, row 64 den.  Non-last blocks drain
                        # PSUM into SBUF first (2 DVE copies) so the next
                        # block's first attn@v write isn't gated on the
                        # whole reciprocal chain; the last block normalizes
                        # straight from PSUM (shorter critical chain).
                        sbo = p3.tile([128, 1024], BF16, tag="sbo", bufs=2,
                                      name="sbo")
                        if last:
                            nc.vector.tensor_copy(sbo[64:65, 0:512],
                                                  pohA[64:65, :])
                            nc.vector.tensor_copy(sbo[64:65, 512:1024],
                                                  pohB[64:65, :])
                            srcA, srcB = pohA[0:64, :], pohB[0:64, :]
                        else:
                            nc.vector.tensor_copy(sbo[0:65, 0:512],
                                                  pohA[0:65, :])
                            nc.vector.tensor_copy(sbo[0:65, 512:1024],
                                                  pohB[0:65, :])
                            srcA = sbo[0:64, 0:512]
                            srcB = sbo[0:64, 512:1024]
                        pb = ps3.tile([128, 1024], F32, tag="aux1024",
                                      bufs=1, name="pb")
                        nc.tensor.matmul(pb[0:64, 0:512],
                                         ones[64:65, 0:64],
                                         sbo[64:65, 0:512], start=True,
                                         stop=True)
                        nc.tensor.matmul(pb[0:64, 512:1024],
                                         ones[64:65, 0:64],
                                         sbo[64:65, 512:1024], start=True,
                                         stop=True)
                        rd = p3.tile([128, 1024], F32, tag="rd", bufs=2,
                                     name="rd")
                        nc.vector.reciprocal_approx_fast(rd[0:64, :],
                                                         pb[0:64, :])
                        tmpB = p3.tile([128, 512], BF16, tag="tmpB",
                                       bufs=2, name="tmpB")
                        nc.vector.tensor_mul(tmpB[0:64, :],
                                             srcB, rd[0:64, 512:1024])
                        nc.sync.dma_start(onorm[p][64:128, nsl],
                                          tmpB[0:64, :])
                        nc.vector.tensor_mul(onorm[p][0:64, nsl],
                                             srcA, rd[0:64, 0:512])
                    return den_norm

                for bi in range(NB * PAIRS):
                    nh, p = bi // PAIRS, bi % PAIRS
                    nsl = slice(nh * 512, (nh + 1) * 512)
                    qt, kt = qk[p], qk[2 + p]
                    # allocated lazily inside av(mt=0) so buffer rotation
                    # follows emission order (avs are popped deferred)
                    poh = {}
                    for mt in range(NT):
                        mtsl = slice(mt * 128, (mt + 1) * 128)
                        psS = ps3.tile([128, 1024], F32, tag="psS", bufs=2,
                                       name="psS")
                        nc.tensor.matmul(psS[:, 0:512], kt[0:64, mtsl],
                                         qt[0:64, nsl], start=True,
                                         stop=True)
                        nc.tensor.matmul(psS[:, 512:1024], kt[64:128, mtsl],
                                         qt[64:128, nsl], start=True,
                                         stop=True)
                        eS = p3.tile([128, 1024], BF16, tag="eS", bufs=2,
                                     name="eS")
                        nc.scalar.activation(eS[:], psS[:], Act.Exp,
                                             scale=float(SCALE))
                        pop_pend()
                        if proj_pend and mt in (4, 9, 14):
                            emit_proj_tile(proj_pend.pop(0))

                        def av(eS=eS, poh=poh, mt=mt, p=p):
                            if mt == 0:
                                poh['A'] = ps3.tile([128, 512], F32,
                                                    tag="pohA", bufs=1,
                                                    name="pohA")
                                poh['B'] = ps3.tile([128, 512], F32,
                                                    tag="pohB", bufs=1,
                                                    name="pohB")
                            base = mt * VSTR + p * 130
                            first, last = (mt == 0), (mt == NT - 1)
                            nc.tensor.matmul(
                                poh['A'][0:65, :], v_sb[:, base:base + 65],
                                eS[:, 0:512], start=first, stop=last)
                            nc.tensor.matmul(
                                poh['B'][0:65, :],
                                v_sb[:, base + 65:base + 130],
                                eS[:, 512:1024], start=first, stop=last)
                        pend.append(('av', av))
                    pend.append(('den', make_den_norm(
                        poh, p, nh, last=(bi == NB * PAIRS - 1))))
                    if p == PAIRS - 1:
                        proj_pend.extend(range(nh * 4, (nh + 1) * 4))
                while pend:
                    pend.pop(0)[1]()
                for nt in proj_pend:
                    emit_proj_tile(nt, tag="psS", bufs=2, split=True)

    nc.compile()
    return nc


def _prep_core_inputs(x, W_qkv, q_gamma, k_gamma, W_proj):
    """Host-side sharding + layout prep. Returns list of 8 in_maps."""
    import ml_dtypes
    f32 = np.float32
    bf16 = np.dtype(ml_dtypes.bfloat16)
    blkdiag = np.kron(np.eye(2, dtype=f32), np.ones((64, 64), f32))
    g2q = np.tile(q_gamma, 2).astype(f32)
    g2k = np.tile(k_gamma, 2).astype(f32)
    ssq_q = (blkdiag * (1.0 / (64.0 * g2q * g2q))[:, None]).astype(bf16)
    ssq_k = (blkdiag * (1.0 / (64.0 * g2k * g2k))[:, None]).astype(bf16)
    in_maps = []
    for core in range(N_CORES):
        b, g = core // 4, core % 4
        heads = [4 * g + j for j in range(HEADS_PER_CORE)]
        qcols, kcols, vcols = [], [], []
        for h in heads:
            wq = W_qkv[h * HD:(h + 1) * HD, :]
            wq = (wq - wq.mean(axis=0, keepdims=True)) * q_gamma[:, None]
            qcols.append(wq.T)
            wk = W_qkv[DIM + h * HD:DIM + (h + 1) * HD, :]
            wk = (wk - wk.mean(axis=0, keepdims=True)) * k_gamma[:, None]
            kcols.append(wk.T)
            vcols.append(W_qkv[2 * DIM + h * HD:2 * DIM + (h + 1) * HD, :].T)
        # pre-tile everything so each DMA line is 2-4KB contiguous:
        # [p, cc*F + o] = W[cc*128 + p, o]
        wqk = np.concatenate(qcols + kcols, axis=1)          # [1024, 512]
        wqk = np.ascontiguousarray(
            wqk.reshape(CC, 128, 512).transpose(1, 0, 2)
            .reshape(128, CC * 512)).astype(bf16)
        wv = np.concatenate(vcols, axis=1)                   # [1024, 256]
        wv = np.ascontiguousarray(
            wv.reshape(CC, 128, 256).transpose(1, 0, 2)
            .reshape(128, CC * 256)).astype(bf16)
        wp = W_proj[:, heads[0] * HD:(heads[-1] + 1) * HD].T  # [256, 1024]
        wp = np.ascontiguousarray(
            wp.reshape(2, 128, DIM).transpose(1, 0, 2)
            .reshape(128, 2 * DIM)).astype(bf16)
        # xT tiled: [p, nch*4096 + cc*512 + j] = x[b][nch*512 + j, cc*128+p]
        xt = x[b].T.reshape(CC, 128, NCH, 512).transpose(1, 2, 0, 3)
        xt = np.ascontiguousarray(
            xt.reshape(128, NCH * CC * 512)).astype(bf16)
        in_maps.append({
            "xT": xt,
            "wqk": wqk, "wv": wv, "wp": wp,
            "ssq_q": ssq_q, "ssq_k": ssq_k,
            "ones": np.ones((128, 64), bf16),
            "ident": np.eye(128, dtype=f32),
        })
    return in_maps


def _numpy_fallback(x, W_qkv, q_gamma, q_beta, k_gamma, k_beta, W_proj, b_proj):
    def ln(t, gamma, beta):
        mu = t.mean(-1, keepdims=True)
        var = ((t - mu) ** 2).mean(-1, keepdims=True)
        return (t - mu) / np.sqrt(var + EPS) * gamma + beta
    Bs, Ns, C = x.shape
    qkv = np.einsum('bnc,oc->bno', x, W_qkv)
    qkv = qkv.reshape(Bs, Ns, 3, HEADS, HD).transpose(2, 0, 3, 1, 4)
    q, k, v = ln(qkv[0], q_gamma, q_beta), ln(qkv[1], k_gamma, k_beta), qkv[2]
    s = np.einsum('bhnd,bhmd->bhnm', q * SCALE, k)
    s = np.exp(s - s.max(-1, keepdims=True))
    p = s / s.sum(-1, keepdims=True)
    o = np.einsum('bhnm,bhmd->bhnd', p, v)
    o = o.transpose(0, 2, 1, 3).reshape(Bs, Ns, C)
    return (np.einsum('bnc,oc->bno', o, W_proj) + b_proj).astype(np.float32)


def kernel(x, W_qkv, q_gamma, q_beta, k_gamma, k_beta, W_proj, b_proj):
    x = np.asarray(x, np.float32)
    W_qkv = np.asarray(W_qkv, np.float32)
    q_gamma = np.asarray(q_gamma, np.float32)
    q_beta = np.asarray(q_beta, np.float32)
    k_gamma = np.asarray(k_gamma, np.float32)
    k_beta = np.asarray(k_beta, np.float32)
    W_proj = np.asarray(W_proj, np.float32)
    b_proj = np.asarray(b_proj, np.float32)

    if np.any(q_beta != 0) or np.any(k_beta != 0):
        # beta terms are not wired into the device kernel (reference always
        # uses beta = 0); fall back to exact host computation
        return _numpy_fallback(x, W_qkv, q_gamma, q_beta, k_gamma, k_beta,
                               W_proj, b_proj)

    from concourse import bass_utils

    if "prog" not in _prog_cache:
        _prog_cache["prog"] = _build_program()
    nc = _prog_cache["prog"]

    in_maps = _prep_core_inputs(x, W_qkv, q_gamma, k_gamma, W_proj)
    res = bass_utils.run_bass_kernel_spmd(nc, in_maps, list(range(N_CORES)))

    out = np.empty((B, N, DIM), np.float32)
    for b in range(B):
        acc = res.results[4 * b + 0]["y"].astype(np.float32).copy()
        for g in range(1, 4):
            acc += res.results[4 * b + g]["y"]
        out[b] = acc + b_proj
    return out


# revision 30
# speedup vs baseline: 1.0031x; 1.0031x over previous
"""Trainium2 Bass kernel for qk-layernorm attention (dense transformer block).

Sharding: 8 cores = 2 batches x 4 head-groups (4 heads each).  Each core
computes qkv projection (its heads only), qk-layernorm, attention, and a
partial output projection for its head slice; the host sums the 4 partials
per batch and adds b_proj.

Key speed choices vs the v1 kernel:
 - Wq/Wk head-slices are mean-centered on the host, so projected q/k have
   exactly zero mean over head-dim: the LN mean path (stats matmul, square,
   subtract) disappears; only E[q^2] remains (one matmul per pair/type).
 - rsqrt via ACT Sqrt + DVE reciprocal_approx_fast (the plain DVE
   reciprocal is ~9ns/elem and dominated the old DVE timeline).
 - Attention runs as one continuous software-pipelined stream over
   (n-block 512, pair, m-tile): PE does S(g) then attn@v(g-1) while ACT
   does exp(g-1); psS is double-buffered (2x2 PSUM banks) so PE never
   waits on the exp read.  Out-projection tiles of block i inject into
   block i+1's stream to fill PE gaps.
 - Softmax denominator: ones-column in the attn@v weights accumulates
   sum(exp) in an extra PSUM row; a K=1 matmul broadcasts both heads'
   denominator rows into one [128,512] bank, one reciprocal_approx_fast,
   two aligned DVE muls.  No DMA partition-shift: pair-head-1's attn@v
   writes PSUM partitions 63:128 directly ([ones|v1] weights, out AP
   offset 63).
 - Split weight/x DMAs so the first proj matmul starts ~3us in.
"""

import numpy as np

DIM = 1024
HEADS = 16
HD = 64
B = 2
N = 2048
EPS = 1e-6
N_CORES = 8
HEADS_PER_CORE = 4
PAIRS = 2          # head pairs per core
CC = 8             # contraction chunks of 128 over DIM
NT = N // 128      # 16 n/m tiles
NCH = N // 512     # 4 chunks of 512
NB = N // 512      # attention n-blocks of 512
VSTR = 260         # v_sb cols per nt: 2 pairs x [v0|ones|v1|ones] (130 each)
SCALE = HD ** -0.5

_prog_cache = {}


def _build_program():
    import concourse.bass as bass
    import concourse.tile as tile
    from concourse import mybir, bacc

    F32 = mybir.dt.float32
    F32R = mybir.dt.float32r
    BF16 = mybir.dt.bfloat16
    Act = mybir.ActivationFunctionType

    nc = bacc.Bacc("TRN2", target_bir_lowering=False, debug=False,
                   num_devices=N_CORES)

    # ---- DRAM I/O ----
    xT_d = nc.dram_tensor("xT", [128, NCH * CC * 512], BF16, kind="ExternalInput").ap()
    wqk_d = nc.dram_tensor("wqk", [128, CC * 512], BF16, kind="ExternalInput").ap()
    wv_d = nc.dram_tensor("wv", [128, CC * 256], BF16, kind="ExternalInput").ap()
    wp_d = nc.dram_tensor("wp", [128, 2 * DIM], BF16, kind="ExternalInput").ap()
    ssq_q_d = nc.dram_tensor("ssq_q", [128, 128], BF16, kind="ExternalInput").ap()
    ssq_k_d = nc.dram_tensor("ssq_k", [128, 128], BF16, kind="ExternalInput").ap()
    ones_d = nc.dram_tensor("ones", [128, 64], BF16, kind="ExternalInput").ap()
    ident_d = nc.dram_tensor("ident", [128, 128], F32R, kind="ExternalInput").ap()
    y_d = nc.dram_tensor("y", [N, DIM], F32, kind="ExternalOutput").ap()


    with tile.TileContext(nc) as tc:
        with tc.tile_pool(name="wts", bufs=1) as wts, \
             tc.tile_pool(name="persist", bufs=1) as persist:
            # 2-cc granular weight tiles so the first matmul gates on
            # only wqk cc0-1 + x cc0-1 (0.5MB)
            wqk_t = [wts.tile([128, 1024], BF16, tag=f"wqk{k}",
                              name=f"wqk{k}") for k in range(4)]
            wv_t = [wts.tile([128, 512], BF16, tag=f"wv{k}",
                             name=f"wv{k}") for k in range(4)]
            nc.sync.dma_start(wqk_t[0][:], wqk_d[:, 0:1024])

            # persistent SBUF tensors
            qk = [persist.tile([128, N], BF16, tag=f"qk{i}", name=f"qk{i}")
                  for i in range(4)]           # 0,1: q pairs; 2,3: k pairs
            vT = [persist.tile([128, N], F32R, tag=f"vT{p}", name=f"vT{p}")
                  for p in range(PAIRS)]
            v_sb = persist.tile([128, NT * VSTR], BF16)
            onorm = [persist.tile([128, N], BF16, tag=f"on{p}", name=f"on{p}")
                     for p in range(PAIRS)]

            wp = wts.tile([128, 2 * DIM], BF16)
            ssq = [wts.tile([128, 128], BF16, tag=f"ssq{t}", name=f"ssq{t}")
                   for t in range(2)]
            ones = wts.tile([128, 64], BF16)
            ident = wts.tile([128, 128], F32R)
            epsb = wts.tile([128, 1], F32)

            def misc_dmas():
                nc.sync.dma_start(wp[:], wp_d[:])
                nc.sync.dma_start(ssq[0][:], ssq_q_d[:])
                nc.sync.dma_start(ssq[1][:], ssq_k_d[:])
                nc.sync.dma_start(ones[:], ones_d[:])
                nc.sync.dma_start(ident[:], ident_d[:])
                # ones columns of v_sb (den accumulators): col 64 of each
                # 65-wide [v | ones] block
                v_ones = v_sb[:].rearrange(
                    "p (nt q c) -> p nt q c", q=4, c=65)[:, :, :, 64:65]
                nc.sync.dma_start(
                    v_ones,
                    ones_d.rearrange("p (nt q) -> p nt q",
                                     q=4)[:, :, :, None])
                nc.gpsimd.memset(epsb[:], EPS)

            # ================= Phase 1: qkv projection + qk-LN ============
            # jobs: (dest tile, kind, col offset within per-cc weight block)
            jobs = [(qk[0], 'qk', 0), (qk[1], 'qk', 128),
                    (qk[2], 'qk', 256), (qk[3], 'qk', 384),
                    (vT[0], 'v', 0), (vT[1], 'v', 128)]

            with tc.tile_pool(name="xp", bufs=2) as xp, \
                 tc.tile_pool(name="t1", bufs=2) as t1p, \
                 tc.tile_pool(name="ps1", bufs=1, space="PSUM") as ps1:

                T_pend = []   # deferred v-transpose closures

                def proj_mms(nch):
                    sl = slice(nch * 512, (nch + 1) * 512)
                    xb = nch * CC * 512
                    # x tiles at 2-cc granularity; for the first chunk,
                    # interleave x / wqk / wv loads so early cc's of all
                    # three streams arrive first
                    xt_t = []
                    for k in range(4):
                        xt = xp.tile([128, 1024], BF16, tag=f"xt{k}",
                                     name=f"xt{k}")
                        nc.sync.dma_start(
                            xt[:], xT_d[:, xb + k * 1024:xb + (k + 1) * 1024])
                        xt_t.append(xt)
                        if nch == 0:
                            if k > 0:
                                nc.sync.dma_start(
                                    wqk_t[k][:],
                                    wqk_d[:, k * 1024:(k + 1) * 1024])
                            nc.sync.dma_start(
                                wv_t[k][:], wv_d[:, k * 512:(k + 1) * 512])
                    accs = [ps1.tile([128, 512], F32, tag=f"acc{j}",
                                     name=f"acc{j}") for j in range(6)]

                    def one_mm(cc, j):
                        dest, kind, off = jobs[j]
                        xsl = xt_t[cc // 2][:, (cc % 2) * 512:
                                            (cc % 2 + 1) * 512]
                        if kind == 'qk':
                            wsl = wqk_t[cc // 2][:, (cc % 2) * 512 + off:
                                                 (cc % 2) * 512 + off + 128]
                        else:
                            wsl = wv_t[cc // 2][:, (cc % 2) * 256 + off:
                                                (cc % 2) * 256 + off + 128]
                        nc.tensor.matmul(accs[j][:], wsl, xsl,
                                         start=(cc == 0),
                                         stop=(cc == CC - 1))

                    if nch == 0:
                        # qk jobs first: first matmuls gate only on
                        # wqkA+xtA while wv streams in
                        for cc in range(CC):
                            for j in range(4):
                                one_mm(cc, j)
                        for cc in range(CC):
                            for j in (4, 5):
                                one_mm(cc, j)
                    else:
                        for cc in range(CC):
                            for j in range(6):
                                one_mm(cc, j)
                            # fill PE slack with deferred v-transposes
                            # (cc<=5: keep the aux psum tag clear for psq)
                            if 2 <= cc <= 5:
                                for _ in range(2):
                                    if T_pend:
                                        T_pend.pop(0)()
                    # drain PSUM -> SBUF, split across DVE and ACT
                    for j, (dest, kind, off) in enumerate(jobs):
                        if j % 2 == 0:
                            nc.vector.tensor_copy(dest[:, sl], accs[j][:])
                        else:
                            nc.scalar.copy(dest[:, sl], accs[j][:])

                def stats_tail(nch):
                    # E[q^2] -> rsqrt -> scale, per (pair, q/k); plus v
                    # transposes into v_sb for this n-chunk
                    sl = slice(nch * 512, (nch + 1) * 512)
                    for p in range(PAIRS):
                        for t in range(2):      # 0 = q, 1 = k
                            src = qk[2 * t + p]
                            sqc = t1p.tile([128, 512], BF16, tag="sqc",
                                           name="sqc")
                            nc.gpsimd.tensor_mul(sqc[:], src[:, sl],
                                                 src[:, sl])
                            psq = ps1.tile([128, 512], F32, tag="aux",
                                           bufs=2, name="psq")
                            nc.tensor.matmul(psq[:], ssq[t][:], sqc[:],
                                             start=True, stop=True)
                            sd = t1p.tile([128, 512], F32, tag="sd",
                                          name="sd")
                            nc.scalar.activation(sd[:], psq[:], Act.Sqrt,
                                                 bias=epsb[:])
                            rs = t1p.tile([128, 512], F32, tag="rs",
                                          name="rs")
                            nc.vector.reciprocal_approx_fast(rs[:], sd[:])
                            nc.vector.tensor_mul(src[:, sl], src[:, sl],
                                                 rs[:])
                    for j in range(4):
                        nt = nch * 4 + j
                        for p in range(PAIRS):
                            def T(nt=nt, p=p, tag="aux", bufs=2,
                                  eng=None):
                                ptr = ps1.tile([128, 128], F32R, tag=tag,
                                               bufs=bufs, name="ptr")
                                nc.tensor.transpose(
                                    ptr[:],
                                    vT[p][:, nt * 128:(nt + 1) * 128],
                                    ident[:])
                                base = nt * VSTR + p * 130
                                dst = v_sb[:, base:base + 130].rearrange(
                                    "p (b c) -> p b c", c=65)[:, :, 0:64]
                                srcv = ptr[:].rearrange(
                                    "p (b c) -> p b c", c=64)
                                if eng == 'vector':
                                    nc.vector.tensor_copy(dst, srcv)
                                else:
                                    nc.scalar.copy(dst, srcv)
                            T_pend.append(T)

                for nch in range(NCH):
                    proj_mms(nch)
                    if nch == 0:
                        misc_dmas()
                    if nch > 0:
                        stats_tail(nch - 1)
                stats_tail(NCH - 1)
                # remaining transposes fill the PE window while the last
                # k-chunk's LN chain completes on ACT/DVE; rotate over the
                # freed acc banks so ptr copies never stall the PE
                ti = 0
                while T_pend:
                    T_pend.pop(0)(tag=f"acc{ti % 6}", bufs=1,
                                  eng='vector' if ti % 2 else 'scalar')
                    ti += 1

            # ================= Phase 3: attention =================
            with tc.tile_pool(name="p3", bufs=1) as p3, \
                 tc.tile_pool(name="ps3", bufs=1, space="PSUM") as ps3:

                pend = []        # deferred closures: ('av'|'den', fn)
                proj_pend = []   # nt tiles ready for out-projection

                def pop_pend():
                    # one av per iteration keeps the software-pipeline lag
                    # constant; block-end den closures ride along right
                    # after the av they follow
                    if pend:
                        pend.pop(0)[1]()
                    while pend and pend[0][0] == 'den':
                        pend.pop(0)[1]()

                def emit_proj_tile(nt, tag="aux1024", bufs=1, split=False):
                    py = ps3.tile([128, 1024], F32, tag=tag, bufs=bufs,
                                  name="py")
                    yt = p3.tile([128, 1024], F32, tag="yt", bufs=3,
                                 name="yt")
                    for oc in range(2):
                        for p in range(PAIRS):
                            nc.tensor.matmul(
                                py[:, oc * 512:(oc + 1) * 512],
                                onorm[p][:, nt * 128:(nt + 1) * 128],
                                wp[:, p * 1024 + oc * 512:
                                   p * 1024 + (oc + 1) * 512],
                                start=(p == 0), stop=(p == PAIRS - 1))
                        if split:
                            # drain per oc-half so the last tile's copy and
                            # DMA overlap the other half's matmuls
                            osl = slice(oc * 512, (oc + 1) * 512)
                            nc.vector.tensor_copy(yt[:, osl], py[:, osl])
                            nc.sync.dma_start(
                                y_d[nt * 128:(nt + 1) * 128, osl],
                                yt[:, osl])
                    if not split:
                        nc.vector.tensor_copy(yt[:], py[:])
                        nc.sync.dma_start(y_d[nt * 128:(nt + 1) * 128, :],
                                          yt[:])

                def make_den_norm(poh, p, nh, last=False):
                    nsl = slice(nh * 512, (nh + 1) * 512)

                    def den_norm():
                        pohA, pohB = poh['A'], poh['B']
                        # rows 0:64 out, row 64 den.  Non-last blocks drain
                        # PSUM into SBUF first (2 DVE copies) so the next
                        # block's first attn@v write isn't gated on the
                        # whole reciprocal chain; the last block normalizes
                        # straight from PSUM (shorter critical chain).
                        sbo = p3.tile([128, 1024], BF16, tag="sbo", bufs=2,
                                      name="sbo")
                        if last:
                            nc.vector.tensor_copy(sbo[64:65, 0:512],
                                                  pohA[64:65, :])
                            nc.vector.tensor_copy(sbo[64:65, 512:1024],
                                                  pohB[64:65, :])
                            srcA, srcB = pohA[0:64, :], pohB[0:64, :]
                        else:
                            nc.vector.tensor_copy(sbo[0:65, 0:512],
                                                  pohA[0:65, :])
                            nc.vector.tensor_copy(sbo[0:65, 512:1024],
                                                  pohB[0:65, :])
                            srcA = sbo[0:64, 0:512]
                            srcB = sbo[0:64, 512:1024]
                        pb = ps3.tile([128, 1024], F32, tag="aux1024",
                                      bufs=1, name="pb")
                        nc.tensor.matmul(pb[0:64, 0:512],
                                         ones[64:65, 0:64],
                                         sbo[64:65, 0:512], start=True,
                                         stop=True)
                        nc.tensor.matmul(pb[0:64, 512:1024],
                                         ones[64:65, 0:64],
                                         sbo[64:65, 512:1024], start=True,
                                         stop=True)
                        rd = p3.tile([128, 1024], F32, tag="rd", bufs=2,
                                     name="rd")
                        nc.vector.reciprocal_approx_fast(rd[0:64, :],
                                                         pb[0:64, :])
                        tmpB = p3.tile([128, 512], BF16, tag="tmpB",
                                       bufs=2, name="tmpB")
                        nc.vector.tensor_mul(tmpB[0:64, :],
                                             srcB, rd[0:64, 512:1024])
                        nc.sync.dma_start(onorm[p][64:128, nsl],
                                          tmpB[0:64, :])
                        nc.vector.tensor_mul(onorm[p][0:64, nsl],
                                             srcA, rd[0:64, 0:512])
                    return den_norm

                for bi in range(NB * PAIRS):
                    nh, p = bi // PAIRS, bi % PAIRS
                    nsl = slice(nh * 512, (nh + 1) * 512)
                    qt, kt = qk[p], qk[2 + p]
                    # allocated lazily inside av(mt=0) so buffer rotation
                    # follows emission order (avs are popped deferred)
                    poh = {}
                    for mt in range(NT):
                        mtsl = slice(mt * 128, (mt + 1) * 128)
                        psS = ps3.tile([128, 1024], F32, tag="psS", bufs=2,
                                       name="psS")
                        nc.tensor.matmul(psS[:, 0:512], kt[0:64, mtsl],
                                         qt[0:64, nsl], start=True,
                                         stop=True)
                        nc.tensor.matmul(psS[:, 512:1024], kt[64:128, mtsl],
                                         qt[64:128, nsl], start=True,
                                         stop=True)
                        eS = p3.tile([128, 1024], BF16, tag="eS", bufs=2,
                                     name="eS")
                        nc.scalar.activation(eS[:], psS[:], Act.Exp,
                                             scale=float(SCALE))
                        pop_pend()
                        if proj_pend and mt in (4, 9, 14):
                            emit_proj_tile(proj_pend.pop(0))

                        def av(eS=eS, poh=poh, mt=mt, p=p):
                            if mt == 0:
                                poh['A'] = ps3.tile([128, 512], F32,
                                                    tag="pohA", bufs=1,
                                                    name="pohA")
                                poh['B'] = ps3.tile([128, 512], F32,
                                                    tag="pohB", bufs=1,
                                                    name="pohB")
                            base = mt * VSTR + p * 130
                            first, last = (mt == 0), (mt == NT - 1)
                            nc.tensor.matmul(
                                poh['A'][0:65, :], v_sb[:, base:base + 65],
                                eS[:, 0:512], start=first, stop=last)
                            nc.tensor.matmul(
                                poh['B'][0:65, :],
                                v_sb[:, base + 65:base + 130],
                                eS[:, 512:1024], start=first, stop=last)
                        pend.append(('av', av))
                    pend.append(('den', make_den_norm(
                        poh, p, nh, last=(bi == NB * PAIRS - 1))))
                    if p == PAIRS - 1:
                        proj_pend.extend(range(nh * 4, (nh + 1) * 4))
                while pend:
                    pend.pop(0)[1]()
                for nt in proj_pend:
                    emit_proj_tile(nt, tag="psS", bufs=2, split=True)

    nc.compile()
    return nc


def _prep_core_inputs(x, W_qkv, q_gamma, k_gamma, W_proj):
    """Host-side sharding + layout prep. Returns list of 8 in_maps."""
    import ml_dtypes
    f32 = np.float32
    bf16 = np.dtype(ml_dtypes.bfloat16)
    blkdiag = np.kron(np.eye(2, dtype=f32), np.ones((64, 64), f32))
    g2q = np.tile(q_gamma, 2).astype(f32)
    g2k = np.tile(k_gamma, 2).astype(f32)
    ssq_q = (blkdiag * (1.0 / (64.0 * g2q * g2q))[:, None]).astype(bf16)
    ssq_k = (blkdiag * (1.0 / (64.0 * g2k * g2k))[:, None]).astype(bf16)
    in_maps = []
    for core in range(N_CORES):
        b, g = core // 4, core % 4
        heads = [4 * g + j for j in range(HEADS_PER_CORE)]
        qcols, kcols, vcols = [], [], []
        for h in heads:
            wq = W_qkv[h * HD:(h + 1) * HD, :]
            wq = (wq - wq.mean(axis=0, keepdims=True)) * q_gamma[:, None]
            qcols.append(wq.T)
            wk = W_qkv[DIM + h * HD:DIM + (h + 1) * HD, :]
            wk = (wk - wk.mean(axis=0, keepdims=True)) * k_gamma[:, None]
            kcols.append(wk.T)
            vcols.append(W_qkv[2 * DIM + h * HD:2 * DIM + (h + 1) * HD, :].T)
        # pre-tile everything so each DMA line is 2-4KB contiguous:
        # [p, cc*F + o] = W[cc*128 + p, o]
        wqk = np.concatenate(qcols + kcols, axis=1)          # [1024, 512]
        wqk = np.ascontiguousarray(
            wqk.reshape(CC, 128, 512).transpose(1, 0, 2)
            .reshape(128, CC * 512)).astype(bf16)
        wv = np.concatenate(vcols, axis=1)                   # [1024, 256]
        wv = np.ascontiguousarray(
            wv.reshape(CC, 128, 256).transpose(1, 0, 2)
            .reshape(128, CC * 256)).astype(bf16)
        wp = W_proj[:, heads[0] * HD:(heads[-1] + 1) * HD].T  # [256, 1024]
        wp = np.ascontiguousarray(
            wp.reshape(2, 128, DIM).transpose(1, 0, 2)
            .reshape(128, 2 * DIM)).astype(bf16)
        # xT tiled: [p, nch*4096 + cc*512 + j] = x[b][nch*512 + j, cc*128+p]
        xt = x[b].T.reshape(CC, 128, NCH, 512).transpose(1, 2, 0, 3)
        xt = np.ascontiguousarray(
            xt.reshape(128, NCH * CC * 512)).astype(bf16)
        in_maps.append({
            "xT": xt,
            "wqk": wqk, "wv": wv, "wp": wp,
            "ssq_q": ssq_q, "ssq_k": ssq_k,
            "ones": np.ones((128, 64), bf16),
            "ident": np.eye(128, dtype=f32),
        })
    return in_maps


def _numpy_fallback(x, W_qkv, q_gamma, q_beta, k_gamma, k_beta, W_proj, b_proj):
    def ln(t, gamma, beta):
        mu = t.mean(-1, keepdims=True)
        var = ((t - mu) ** 2).mean(-1, keepdims=True)
        return (t - mu) / np.sqrt(var + EPS) * gamma + beta
    Bs, Ns, C = x.shape
    qkv = np.einsum('bnc,oc->bno', x, W_qkv)
    qkv = qkv.reshape(Bs, Ns, 3, HEADS, HD).transpose(2, 0, 3, 1, 4)
    q, k, v = ln(qkv[0], q_gamma, q_beta), ln(qkv[1], k_gamma, k_beta), qkv[2]
    s = np.einsum('bhnd,bhmd->bhnm', q * SCALE, k)
    s = np.exp(s - s.max(-1, keepdims=True))
    p = s / s.sum(-1, keepdims=True)
    o = np.einsum('bhnm,bhmd->bhnd', p, v)
    o = o.transpose(0, 2, 1, 3).reshape(Bs, Ns, C)
    return (np.einsum('bnc,oc->bno', o, W_proj) + b_proj).astype(np.float32)


def kernel(x, W_qkv, q_gamma, q_beta, k_gamma, k_beta, W_proj, b_proj):
    x = np.asarray(x, np.float32)
    W_qkv = np.asarray(W_qkv, np.float32)
    q_gamma = np.asarray(q_gamma, np.float32)
    q_beta = np.asarray(q_beta, np.float32)
    k_gamma = np.asarray(k_gamma, np.float32)
    k_beta = np.asarray(k_beta, np.float32)
    W_proj = np.asarray(W_proj, np.float32)
    b_proj = np.asarray(b_proj, np.float32)

    if np.any(q_beta != 0) or np.any(k_beta != 0):
        # beta terms are not wired into the device kernel (reference always
        # uses beta = 0); fall back to exact host computation
        return _numpy_fallback(x, W_qkv, q_gamma, q_beta, k_gamma, k_beta,
                               W_proj, b_proj)

    from concourse import bass_utils

    if "prog" not in _prog_cache:
        _prog_cache["prog"] = _build_program()
    nc = _prog_cache["prog"]

    in_maps = _prep_core_inputs(x, W_qkv, q_gamma, k_gamma, W_proj)
    res = bass_utils.run_bass_kernel_spmd(nc, in_maps, list(range(N_CORES)))

    out = np.empty((B, N, DIM), np.float32)
    for b in range(B):
        acc = res.results[4 * b + 0]["y"].astype(np.float32).copy()
        for g in range(1, 4):
            acc += res.results[4 * b + g]["y"]
        out[b] = acc + b_proj
    return out


# revision 31
# speedup vs baseline: 1.0175x; 1.0144x over previous
"""Trainium2 Bass kernel for qk-layernorm attention (dense transformer block).

Sharding: 8 cores = 2 batches x 4 head-groups (4 heads each).  Each core
computes qkv projection (its heads only), qk-layernorm, attention, and a
partial output projection for its head slice; the host sums the 4 partials
per batch and adds b_proj.

Key speed choices vs the v1 kernel:
 - Wq/Wk head-slices are mean-centered on the host, so projected q/k have
   exactly zero mean over head-dim: the LN mean path (stats matmul, square,
   subtract) disappears; only E[q^2] remains (one matmul per pair/type).
 - rsqrt via ACT Sqrt + DVE reciprocal_approx_fast (the plain DVE
   reciprocal is ~9ns/elem and dominated the old DVE timeline).
 - Attention runs as one continuous software-pipelined stream over
   (n-block 512, pair, m-tile): PE does S(g) then attn@v(g-1) while ACT
   does exp(g-1); psS is double-buffered (2x2 PSUM banks) so PE never
   waits on the exp read.  Out-projection tiles of block i inject into
   block i+1's stream to fill PE gaps.
 - Softmax denominator: ones-column in the attn@v weights accumulates
   sum(exp) in an extra PSUM row; a K=1 matmul broadcasts both heads'
   denominator rows into one [128,512] bank, one reciprocal_approx_fast,
   two aligned DVE muls.  No DMA partition-shift: pair-head-1's attn@v
   writes PSUM partitions 63:128 directly ([ones|v1] weights, out AP
   offset 63).
 - Split weight/x DMAs so the first proj matmul starts ~3us in.
"""

import numpy as np

DIM = 1024
HEADS = 16
HD = 64
B = 2
N = 2048
EPS = 1e-6
N_CORES = 8
HEADS_PER_CORE = 4
PAIRS = 2          # head pairs per core
CC = 8             # contraction chunks of 128 over DIM
NT = N // 128      # 16 n/m tiles
NCH = N // 512     # 4 chunks of 512
NB = N // 512      # attention n-blocks of 512
VSTR = 260         # v_sb cols per nt: 2 pairs x [v0|ones|v1|ones] (130 each)
SCALE = HD ** -0.5

_prog_cache = {}


def _build_program():
    import concourse.bass as bass
    import concourse.tile as tile
    from concourse import mybir, bacc

    F32 = mybir.dt.float32
    F32R = mybir.dt.float32r
    BF16 = mybir.dt.bfloat16
    Act = mybir.ActivationFunctionType

    nc = bacc.Bacc("TRN2", target_bir_lowering=False, debug=False,
                   num_devices=N_CORES)

    # ---- DRAM I/O ----
    xT_d = nc.dram_tensor("xT", [128, NCH * CC * 512], BF16, kind="ExternalInput").ap()
    wqk_d = nc.dram_tensor("wqk", [128, CC * 512], BF16, kind="ExternalInput").ap()
    wv_d = nc.dram_tensor("wv", [128, CC * 256], BF16, kind="ExternalInput").ap()
    wp_d = nc.dram_tensor("wp", [128, 2 * DIM], BF16, kind="ExternalInput").ap()
    ssq_q_d = nc.dram_tensor("ssq_q", [128, 128], BF16, kind="ExternalInput").ap()
    ssq_k_d = nc.dram_tensor("ssq_k", [128, 128], BF16, kind="ExternalInput").ap()
    ones_d = nc.dram_tensor("ones", [128, 64], BF16, kind="ExternalInput").ap()
    ident_d = nc.dram_tensor("ident", [128, 128], F32R, kind="ExternalInput").ap()
    y_d = nc.dram_tensor("y", [N, DIM], F32, kind="ExternalOutput").ap()


    with tile.TileContext(nc) as tc:
        with tc.tile_pool(name="wts", bufs=1) as wts, \
             tc.tile_pool(name="persist", bufs=1) as persist:
            # qk-projection weights first (gate the very first matmuls)
            wqkA = wts.tile([128, 4 * 512], BF16)
            nc.sync.dma_start(wqkA[:], wqk_d[:, 0:2048])
            wqkB = wts.tile([128, 4 * 512], BF16)
            wvA = wts.tile([128, 4 * 256], BF16)
            wvB = wts.tile([128, 4 * 256], BF16)

            # persistent SBUF tensors
            qk = [persist.tile([128, N], BF16, tag=f"qk{i}", name=f"qk{i}")
                  for i in range(4)]           # 0,1: q pairs; 2,3: k pairs
            vT = [persist.tile([128, N], F32R, tag=f"vT{p}", name=f"vT{p}")
                  for p in range(PAIRS)]
            v_sb = persist.tile([128, NT * VSTR], BF16)
            onorm = [persist.tile([128, N], BF16, tag=f"on{p}", name=f"on{p}")
                     for p in range(PAIRS)]

            wp = wts.tile([128, 2 * DIM], BF16)
            ssq = [wts.tile([128, 128], BF16, tag=f"ssq{t}", name=f"ssq{t}")
                   for t in range(2)]
            ones = wts.tile([128, 64], BF16)
            ident = wts.tile([128, 128], F32R)
            epsb = wts.tile([128, 1], F32)

            def misc_dmas():
                nc.sync.dma_start(wp[:], wp_d[:])
                nc.sync.dma_start(ssq[0][:], ssq_q_d[:])
                nc.sync.dma_start(ssq[1][:], ssq_k_d[:])
                nc.sync.dma_start(ones[:], ones_d[:])
                nc.sync.dma_start(ident[:], ident_d[:])
                # ones columns of v_sb (den accumulators): col 64 of each
                # 65-wide [v | ones] block
                v_ones = v_sb[:].rearrange(
                    "p (nt q c) -> p nt q c", q=4, c=65)[:, :, :, 64:65]
                nc.sync.dma_start(
                    v_ones,
                    ones_d.rearrange("p (nt q) -> p nt q",
                                     q=4)[:, :, :, None])
                nc.gpsimd.memset(epsb[:], EPS)

            # ================= Phase 1: qkv projection + qk-LN ============
            # jobs: (dest tile, kind, col offset within per-cc weight block)
            jobs = [(qk[0], 'qk', 0), (qk[1], 'qk', 128),
                    (qk[2], 'qk', 256), (qk[3], 'qk', 384),
                    (vT[0], 'v', 0), (vT[1], 'v', 128)]

            with tc.tile_pool(name="xp", bufs=2) as xp, \
                 tc.tile_pool(name="t1", bufs=2) as t1p, \
                 tc.tile_pool(name="ps1", bufs=1, space="PSUM") as ps1:

                T_pend = []   # deferred v-transpose closures

                def proj_mms(nch):
                    sl = slice(nch * 512, (nch + 1) * 512)
                    xb = nch * CC * 512
                    xtA = xp.tile([128, 4 * 512], BF16, tag="xtA", name="xtA")
                    nc.sync.dma_start(xtA[:], xT_d[:, xb:xb + 2048])
                    if nch == 0:
                        # stagger the weight loads so the first matmuls
                        # gate only on wqkA + xtA
                        nc.sync.dma_start(wqkB[:], wqk_d[:, 2048:4096])
                        nc.sync.dma_start(wvA[:], wv_d[:, 0:1024])
                    xtB = xp.tile([128, 4 * 512], BF16, tag="xtB", name="xtB")
                    nc.sync.dma_start(xtB[:], xT_d[:, xb + 2048:xb + 4096])
                    if nch == 0:
                        nc.sync.dma_start(wvB[:], wv_d[:, 1024:2048])
                    accs = [ps1.tile([128, 512], F32, tag=f"acc{j}",
                                     name=f"acc{j}") for j in range(6)]

                    def one_mm(cc, j):
                        dest, kind, off = jobs[j]
                        xt = xtA if cc < 4 else xtB
                        xsl = xt[:, (cc % 4) * 512:(cc % 4 + 1) * 512]
                        if kind == 'qk':
                            w = wqkA if cc < 4 else wqkB
                            wsl = w[:, (cc % 4) * 512 + off:
                                    (cc % 4) * 512 + off + 128]
                        else:
                            w = wvA if cc < 4 else wvB
                            wsl = w[:, (cc % 4) * 256 + off:
                                    (cc % 4) * 256 + off + 128]
                        nc.tensor.matmul(accs[j][:], wsl, xsl,
                                         start=(cc == 0),
                                         stop=(cc == CC - 1))

                    if nch == 0:
                        # qk jobs first: first matmuls gate only on
                        # wqkA+xtA while wv streams in
                        for cc in range(CC):
                            for j in range(4):
                                one_mm(cc, j)
                        for cc in range(CC):
                            for j in (4, 5):
                                one_mm(cc, j)
                    else:
                        for cc in range(CC):
                            for j in range(6):
                                one_mm(cc, j)
                            # fill PE slack with deferred v-transposes
                            # (cc<=5: keep the aux psum tag clear for psq)
                            if 2 <= cc <= 5:
                                for _ in range(2):
                                    if T_pend:
                                        T_pend.pop(0)()
                    # drain PSUM -> SBUF, split across DVE and ACT
                    for j, (dest, kind, off) in enumerate(jobs):
                        if j % 2 == 0:
                            nc.vector.tensor_copy(dest[:, sl], accs[j][:])
                        else:
                            nc.scalar.copy(dest[:, sl], accs[j][:])

                def stats_tail(nch):
                    # E[q^2] -> rsqrt -> scale, per (pair, q/k); plus v
                    # transposes into v_sb for this n-chunk
                    sl = slice(nch * 512, (nch + 1) * 512)
                    for p in range(PAIRS):
                        for t in range(2):      # 0 = q, 1 = k
                            src = qk[2 * t + p]
                            sqc = t1p.tile([128, 512], BF16, tag="sqc",
                                           name="sqc")
                            nc.gpsimd.tensor_mul(sqc[:], src[:, sl],
                                                 src[:, sl])
                            psq = ps1.tile([128, 512], F32, tag="aux",
                                           bufs=2, name="psq")
                            nc.tensor.matmul(psq[:], ssq[t][:], sqc[:],
                                             start=True, stop=True)
                            sd = t1p.tile([128, 512], F32, tag="sd",
                                          name="sd")
                            nc.scalar.activation(sd[:], psq[:], Act.Sqrt,
                                                 bias=epsb[:])
                            rs = t1p.tile([128, 512], F32, tag="rs",
                                          name="rs")
                            nc.vector.reciprocal_approx_fast(rs[:], sd[:])
                            nc.vector.tensor_mul(src[:, sl], src[:, sl],
                                                 rs[:])
                    for j in range(4):
                        nt = nch * 4 + j
                        for p in range(PAIRS):
                            def T(nt=nt, p=p, tag="aux", bufs=2,
                                  eng=None):
                                ptr = ps1.tile([128, 128], F32R, tag=tag,
                                               bufs=bufs, name="ptr")
                                nc.tensor.transpose(
                                    ptr[:],
                                    vT[p][:, nt * 128:(nt + 1) * 128],
                                    ident[:])
                                base = nt * VSTR + p * 130
                                dst = v_sb[:, base:base + 130].rearrange(
                                    "p (b c) -> p b c", c=65)[:, :, 0:64]
                                srcv = ptr[:].rearrange(
                                    "p (b c) -> p b c", c=64)
                                if eng == 'vector':
                                    nc.vector.tensor_copy(dst, srcv)
                                else:
                                    nc.scalar.copy(dst, srcv)
                            T_pend.append(T)

                for nch in range(NCH):
                    proj_mms(nch)
                    if nch == 0:
                        misc_dmas()
                    if nch > 0:
                        stats_tail(nch - 1)
                stats_tail(NCH - 1)
                # remaining transposes fill the PE window while the last
                # k-chunk's LN chain completes on ACT/DVE; rotate over the
                # freed acc banks so ptr copies never stall the PE
                ti = 0
                while T_pend:
                    T_pend.pop(0)(tag=f"acc{ti % 6}", bufs=1,
                                  eng='vector' if ti % 2 else 'scalar')
                    ti += 1

            # ================= Phase 3: attention =================
            with tc.tile_pool(name="p3", bufs=1) as p3, \
                 tc.tile_pool(name="ps3", bufs=1, space="PSUM") as ps3:

                pend = []        # deferred closures: ('av'|'den', fn)
                proj_pend = []   # nt tiles ready for out-projection

                def pop_pend():
                    # one av per iteration keeps the software-pipeline lag
                    # constant; block-end den closures ride along right
                    # after the av they follow
                    if pend:
                        pend.pop(0)[1]()
                    while pend and pend[0][0] == 'den':
                        pend.pop(0)[1]()

                def emit_proj_tile(nt, tag="aux1024", bufs=1, split=False):
                    py = ps3.tile([128, 1024], F32, tag=tag, bufs=bufs,
                                  name="py")
                    yt = p3.tile([128, 1024], F32, tag="yt", bufs=3,
                                 name="yt")
                    for oc in range(2):
                        for p in range(PAIRS):
                            nc.tensor.matmul(
                                py[:, oc * 512:(oc + 1) * 512],
                                onorm[p][:, nt * 128:(nt + 1) * 128],
                                wp[:, p * 1024 + oc * 512:
                                   p * 1024 + (oc + 1) * 512],
                                start=(p == 0), stop=(p == PAIRS - 1))
                        if split:
                            # drain per oc-half so the last tile's copy and
                            # DMA overlap the other half's matmuls
                            osl = slice(oc * 512, (oc + 1) * 512)
                            nc.vector.tensor_copy(yt[:, osl], py[:, osl])
                            nc.sync.dma_start(
                                y_d[nt * 128:(nt + 1) * 128, osl],
                                yt[:, osl])
                    if not split:
                        nc.vector.tensor_copy(yt[:], py[:])
                        nc.sync.dma_start(y_d[nt * 128:(nt + 1) * 128, :],
                                          yt[:])

                def make_den_norm(poh, p, nh, last=False):
                    nsl = slice(nh * 512, (nh + 1) * 512)

                    def den_norm():
                        pohA, pohB = poh['A'], poh['B']
                        # rows 0:64 out, row 64 den.  Non-last blocks drain
                        # PSUM into SBUF first (2 DVE copies) so the next
                        # block's first attn@v write isn't gated on the
                        # whole reciprocal chain; the last block normalizes
                        # straight from PSUM (shorter critical chain).
                        sbo = p3.tile([128, 1024], BF16, tag="sbo", bufs=2,
                                      name="sbo")
                        if last:
                            nc.vector.tensor_copy(sbo[64:65, 0:512],
                                                  pohA[64:65, :])
                            nc.vector.tensor_copy(sbo[64:65, 512:1024],
                                                  pohB[64:65, :])
                            srcA, srcB = pohA[0:64, :], pohB[0:64, :]
                        else:
                            nc.vector.tensor_copy(sbo[0:65, 0:512],
                                                  pohA[0:65, :])
                            nc.vector.tensor_copy(sbo[0:65, 512:1024],
                                                  pohB[0:65, :])
                            srcA = sbo[0:64, 0:512]
                            srcB = sbo[0:64, 512:1024]
                        pb = ps3.tile([128, 1024], F32, tag="aux1024",
                                      bufs=1, name="pb")
                        nc.tensor.matmul(pb[0:64, 0:512],
                                         ones[64:65, 0:64],
                                         sbo[64:65, 0:512], start=True,
                                         stop=True)
                        nc.tensor.matmul(pb[0:64, 512:1024],
                                         ones[64:65, 0:64],
                                         sbo[64:65, 512:1024], start=True,
                                         stop=True)
                        rd = p3.tile([128, 1024], F32, tag="rd", bufs=2,
                                     name="rd")
                        nc.vector.reciprocal_approx_fast(rd[0:64, :],
                                                         pb[0:64, :])
                        tmpB = p3.tile([128, 512], BF16, tag="tmpB",
                                       bufs=2, name="tmpB")
                        nc.vector.tensor_mul(tmpB[0:64, :],
                                             srcB, rd[0:64, 512:1024])
                        nc.sync.dma_start(onorm[p][64:128, nsl],
                                          tmpB[0:64, :])
                        nc.vector.tensor_mul(onorm[p][0:64, nsl],
                                             srcA, rd[0:64, 0:512])
                    return den_norm

                for bi in range(NB * PAIRS):
                    nh, p = bi // PAIRS, bi % PAIRS
                    nsl = slice(nh * 512, (nh + 1) * 512)
                    qt, kt = qk[p], qk[2 + p]
                    # allocated lazily inside av(mt=0) so buffer rotation
                    # follows emission order (avs are popped deferred)
                    poh = {}
                    for mt in range(NT):
                        mtsl = slice(mt * 128, (mt + 1) * 128)
                        psS = ps3.tile([128, 1024], F32, tag="psS", bufs=2,
                                       name="psS")
                        nc.tensor.matmul(psS[:, 0:512], kt[0:64, mtsl],
                                         qt[0:64, nsl], start=True,
                                         stop=True)
                        nc.tensor.matmul(psS[:, 512:1024], kt[64:128, mtsl],
                                         qt[64:128, nsl], start=True,
                                         stop=True)
                        eS = p3.tile([128, 1024], BF16, tag="eS", bufs=2,
                                     name="eS")
                        nc.scalar.activation(eS[:], psS[:], Act.Exp,
                                             scale=float(SCALE))
                        pop_pend()
                        if proj_pend and mt in (4, 9, 14):
                            emit_proj_tile(proj_pend.pop(0))

                        def av(eS=eS, poh=poh, mt=mt, p=p):
                            if mt == 0:
                                poh['A'] = ps3.tile([128, 512], F32,
                                                    tag="pohA", bufs=1,
                                                    name="pohA")
                                poh['B'] = ps3.tile([128, 512], F32,
                                                    tag="pohB", bufs=1,
                                                    name="pohB")
                            base = mt * VSTR + p * 130
                            first, last = (mt == 0), (mt == NT - 1)
                            nc.tensor.matmul(
                                poh['A'][0:65, :], v_sb[:, base:base + 65],
                                eS[:, 0:512], start=first, stop=last)
                            nc.tensor.matmul(
                                poh['B'][0:65, :],
                                v_sb[:, base + 65:base + 130],
                                eS[:, 512:1024], start=first, stop=last)
                        pend.append(('av', av))
                    pend.append(('den', make_den_norm(
                        poh, p, nh, last=(bi == NB * PAIRS - 1))))
                    if p == PAIRS - 1:
                        proj_pend.extend(range(nh * 4, (nh + 1) * 4))
                while pend:
                    pend.pop(0)[1]()
                for nt in proj_pend:
                    emit_proj_tile(nt, tag="psS", bufs=2, split=True)

    nc.compile()
    return nc


def _prep_core_inputs(x, W_qkv, q_gamma, k_gamma, W_proj):
    """Host-side sharding + layout prep. Returns list of 8 in_maps."""
    import ml_dtypes
    f32 = np.float32
    bf16 = np.dtype(ml_dtypes.bfloat16)
    blkdiag = np.kron(np.eye(2, dtype=f32), np.ones((64, 64), f32))
    g2q = np.tile(q_gamma, 2).astype(f32)
    g2k = np.tile(k_gamma, 2).astype(f32)
    ssq_q = (blkdiag * (1.0 / (64.0 * g2q * g2q))[:, None]).astype(bf16)
    ssq_k = (blkdiag * (1.0 / (64.0 * g2k * g2k))[:, None]).astype(bf16)
    in_maps = []
    for core in range(N_CORES):
        b, g = core // 4, core % 4
        heads = [4 * g + j for j in range(HEADS_PER_CORE)]
        qcols, kcols, vcols = [], [], []
        for h in heads:
            wq = W_qkv[h * HD:(h + 1) * HD, :]
            wq = (wq - wq.mean(axis=0, keepdims=True)) * q_gamma[:, None]
            qcols.append(wq.T)
            wk = W_qkv[DIM + h * HD:DIM + (h + 1) * HD, :]
            wk = (wk - wk.mean(axis=0, keepdims=True)) * k_gamma[:, None]
            kcols.append(wk.T)
            vcols.append(W_qkv[2 * DIM + h * HD:2 * DIM + (h + 1) * HD, :].T)
        # pre-tile everything so each DMA line is 2-4KB contiguous:
        # [p, cc*F + o] = W[cc*128 + p, o]
        wqk = np.concatenate(qcols + kcols, axis=1)          # [1024, 512]
        wqk = np.ascontiguousarray(
            wqk.reshape(CC, 128, 512).transpose(1, 0, 2)
            .reshape(128, CC * 512)).astype(bf16)
        wv = np.concatenate(vcols, axis=1)                   # [1024, 256]
        wv = np.ascontiguousarray(
            wv.reshape(CC, 128, 256).transpose(1, 0, 2)
            .reshape(128, CC * 256)).astype(bf16)
        wp = W_proj[:, heads[0] * HD:(heads[-1] + 1) * HD].T  # [256, 1024]
        wp = np.ascontiguousarray(
            wp.reshape(2, 128, DIM).transpose(1, 0, 2)
            .reshape(128, 2 * DIM)).astype(bf16)
        # xT tiled: [p, nch*4096 + cc*512 + j] = x[b][nch*512 + j, cc*128+p]
        xt = x[b].T.reshape(CC, 128, NCH, 512).transpose(1, 2, 0, 3)
        xt = np.ascontiguousarray(
            xt.reshape(128, NCH * CC * 512)).astype(bf16)
        in_maps.append({
            "xT": xt,
            "wqk": wqk, "wv": wv, "wp": wp,
            "ssq_q": ssq_q, "ssq_k": ssq_k,
            "ones": np.ones((128, 64), bf16),
            "ident": np.eye(128, dtype=f32),
        })
    return in_maps


def _numpy_fallback(x, W_qkv, q_gamma, q_beta, k_gamma, k_beta, W_proj, b_proj):
    def ln(t, gamma, beta):
        mu = t.mean(-1, keepdims=True)
        var = ((t - mu) ** 2).mean(-1, keepdims=True)
        return (t - mu) / np.sqrt(var + EPS) * gamma + beta
    Bs, Ns, C = x.shape
    qkv = np.einsum('bnc,oc->bno', x, W_qkv)
    qkv = qkv.reshape(Bs, Ns, 3, HEADS, HD).transpose(2, 0, 3, 1, 4)
    q, k, v = ln(qkv[0], q_gamma, q_beta), ln(qkv[1], k_gamma, k_beta), qkv[2]
    s = np.einsum('bhnd,bhmd->bhnm', q * SCALE, k)
    s = np.exp(s - s.max(-1, keepdims=True))
    p = s / s.sum(-1, keepdims=True)
    o = np.einsum('bhnm,bhmd->bhnd', p, v)
    o = o.transpose(0, 2, 1, 3).reshape(Bs, Ns, C)
    return (np.einsum('bnc,oc->bno', o, W_proj) + b_proj).astype(np.float32)


def kernel(x, W_qkv, q_gamma, q_beta, k_gamma, k_beta, W_proj, b_proj):
    x = np.asarray(x, np.float32)
    W_qkv = np.asarray(W_qkv, np.float32)
    q_gamma = np.asarray(q_gamma, np.float32)
    q_beta = np.asarray(q_beta, np.float32)
    k_gamma = np.asarray(k_gamma, np.float32)
    k_beta = np.asarray(k_beta, np.float32)
    W_proj = np.asarray(W_proj, np.float32)
    b_proj = np.asarray(b_proj, np.float32)

    if np.any(q_beta != 0) or np.any(k_beta != 0):
        # beta terms are not wired into the device kernel (reference always
        # uses beta = 0); fall back to exact host computation
        return _numpy_fallback(x, W_qkv, q_gamma, q_beta, k_gamma, k_beta,
                               W_proj, b_proj)

    from concourse import bass_utils

    if "prog" not in _prog_cache:
        _prog_cache["prog"] = _build_program()
    nc = _prog_cache["prog"]

    in_maps = _prep_core_inputs(x, W_qkv, q_gamma, k_gamma, W_proj)
    res = bass_utils.run_bass_kernel_spmd(nc, in_maps, list(range(N_CORES)))

    out = np.empty((B, N, DIM), np.float32)
    for b in range(B):
        acc = res.results[4 * b + 0]["y"].astype(np.float32).copy()
        for g in range(1, 4):
            acc += res.results[4 * b + g]["y"]
        out[b] = acc + b_proj
    return out
